# revision 15
# baseline (speedup 1.0000x reference)
"""3-layer GAT on 8 Trainium2 NeuronCores (Bass/Tile) — v5.

Key design points (v5 over v4):
  - eq / eqd edge-selection matrices are compile-time constants: built on
    the host, streamed from DRAM per tile (they only depend on the edge
    list). Kills the DVE is_equal builds, the per-chunk PE transposes and
    ~1500 PSUM->SBUF copies of v4.
  - self-loop edges are split out of the dma_gather calls: the self chunk
    of every tile is an identity eq block whose rows come from a direct
    contiguous DMA (L2/L3) or a local xT transform (L1).
  - aggregation uses wf-scaled feature rows (G' = G * wf) so the agg
    matmul has a single stationary operand (eq chunk) instead of per-head
    S matrices; for L1 the scaling is folded into the PSUM->SBUF copy of
    the per-edge transform (free).
  - block AllGathers fire as soon as the producing tiles are stored,
    hiding the collective behind the tail of the producing layer.
  - hs2 rows shrunk 1280B -> 1152B.

Self-contained: only imports the system concourse install.
"""

import os
import sys

for _p in ("/opt/trn_rl_repo", "/root/.axon_site/_ro/trn_rl_repo"):
    if os.path.isdir(_p) and _p not in sys.path:
        sys.path.insert(0, _p)

from dataclasses import dataclass

import ml_dtypes
import numpy as np

import concourse.bacc as bacc
import concourse.bass as bass
import concourse.tile as tile
from concourse import mybir
from concourse.bass_utils import run_bass_kernel_spmd

P = 128
BF16 = mybir.dt.bfloat16
F32 = mybir.dt.float32
I16 = mybir.dt.int16
AL = mybir.AluOpType
AF = mybir.ActivationFunctionType

NEG_SLOPE_ATT = 0.2
NEG_SLOPE_ACT = 0.01
LN_EPS = 1e-5
MAX_GIDX = 896           # dma_gather HW limit: <=~1000 indices per call


def _ceil(a, b):
    return -(-a // b)


def _pad_elem_bytes(n_bf16):
    """bf16 row length (elements) padded so row bytes are a mult of 256."""
    return _ceil(n_bf16 * 2, 256) * 128


@dataclass
class Cfg:
    N: int = 50000
    E: int = 400000
    F_IN: int = 256
    HEADS: int = 4
    C1: int = 256
    C2: int = 128
    NCLS: int = 32
    NCORES: int = 8

    def __post_init__(self):
        assert self.N % self.NCORES == 0
        self.NL = self.N // self.NCORES
        self.T = _ceil(self.NL, P)
        self.NLP = self.T * P
        self.NPTOT = self.NLP * self.NCORES
        self.NG = 2
        self.BLK_T = [25, 24]
        assert sum(self.BLK_T) == self.T
        self.SBT = [0, 25]
        self.GROWS = [b * P * self.NCORES for b in self.BLK_T]
        self.GBASE = [0, self.GROWS[0]]
        assert max(self.GROWS) < 32768  # int16 gather indices per group
        H = self.HEADS
        self.CO1 = H * self.C1
        self.CO2 = H * self.C2
        assert self.F_IN % P == 0 and self.CO1 % P == 0 and self.CO2 % P == 0
        self.ELEM2 = _pad_elem_bytes(self.CO2 + H)    # [h2|s2|pad] rows: 576
        self.ELEM3 = _pad_elem_bytes(self.NCLS + 1)   # [h3|s3|pad] rows: 128
        self.W1w = self.CO1 + 2 * H                   # [W1 | U_s | U_d]
        self.W2w = self.CO2 + 2 * H
        self.W3w = self.NCLS + 2


@dataclass
class Meta:
    nch: list   # [T][NG] gathered chunk counts (excl. self; common per core)
    sig: list   # [NG][T] idx col offsets (group-major)
    sce: list   # [T] eq-table chunk offset for tile t (in chunks)
    ntot: list  # [T] total chunks incl self = 1 + sum(nch[t])
    SI: int
    SCE: int    # total eq chunks


def _grp_map(cfg: Cfg, core, loc):
    """(source block, within-block row index) for node (core, local idx)."""
    t = loc // P
    b = (t >= cfg.BLK_T[0]).astype(np.int64)
    blk_t = np.array(cfg.BLK_T)[b]
    sb = np.array(cfg.SBT)[b]
    return b, core * blk_t * P + (loc - sb * P)


def _gidx_map(cfg: Cfg, core, loc):
    b, off = _grp_map(cfg, core, loc)
    return np.array(cfg.GBASE)[b] + off


def host_prep(cfg: Cfg, x, edge_src, edge_dst,
              W1, a_src1, a_dst1, b1, ln1_g, ln1_b,
              W2, a_src2, a_dst2, b2, ln2_g, ln2_b,
              W3, a_src3, a_dst3, b3, ln3_g, ln3_b):
    c = cfg
    bf = ml_dtypes.bfloat16

    # ---- real edges only (self loops handled via identity chunks)
    src = edge_src.astype(np.int64)
    dst = edge_dst.astype(np.int64)

    dst_core = dst // c.NL
    dstloc = dst - dst_core * c.NL
    tile_id = dstloc // P
    grp, idx16 = _grp_map(c, src // c.NL, src % c.NL)
    grp = grp.astype(np.int64)
    idx16 = idx16.astype(np.int64)
    NG = c.NG

    counts = np.zeros((c.NCORES, c.T, NG), np.int64)
    np.add.at(counts, (dst_core, tile_id, grp), 1)
    nch = np.maximum(_ceil(counts.max(axis=0), P), 0)  # [T,NG] chunks
    assert nch.max() * P <= MAX_GIDX, nch.max()
    sig = np.zeros((NG, c.T), np.int64)
    acc = 0
    for g in range(NG):
        for t in range(c.T):
            sig[g, t] = acc
            acc += int(nch[t, g]) * (P // 16)
    SI = int(acc)
    ntot = [1 + int(nch[t, 0]) + int(nch[t, 1]) for t in range(c.T)]
    sce = np.zeros(c.T, np.int64)
    acc_e = 0
    for t in range(c.T):
        sce[t] = acc_e
        acc_e += ntot[t]
    SCE = int(acc_e)
    meta = Meta(nch=nch.tolist(), sig=sig.tolist(), sce=sce.tolist(),
                ntot=ntot, SI=SI, SCE=SCE)

    order = np.lexsort((grp, tile_id, dst_core))
    src_s = idx16[order]
    dstrel_s = (dstloc - tile_id * P)[order]

    starts = np.zeros((c.NCORES, c.T, NG), np.int64)
    run = 0
    for cc in range(c.NCORES):
        for t in range(c.T):
            for g in range(NG):
                starts[cc, t, g] = run
                run += int(counts[cc, t, g])

    ident128 = np.eye(P, dtype=np.float32)
    idx_tabs, eqA_tabs, eqD_tabs = [], [], []
    for cc in range(c.NCORES):
        itab = np.zeros((16, SI), np.int16)
        eqA = np.zeros((P, SCE * P), np.float32)
        eqD = np.zeros((P, SCE * P), np.float32)
        for t in range(c.T):
            k0 = int(sce[t])
            # chunk 0: identity (self loops)
            eqA[:, k0 * P:(k0 + 1) * P] = ident128
            eqD[:, k0 * P:(k0 + 1) * P] = ident128
            kg = k0 + 1
            for g in range(NG):
                m = int(counts[cc, t, g])
                n = int(nch[t, g])
                if n == 0:
                    continue
                s0 = int(starts[cc, t, g])
                iv = np.zeros(n * P, np.int16)
                iv[:m] = src_s[s0:s0 + m].astype(np.int16)
                cols = int(sig[g, t])
                itab[:, cols:cols + n * (P // 16)] = iv.reshape(
                    n * P // 16, 16).T
                dv = np.full(n * P, -1, np.int64)
                dv[:m] = dstrel_s[s0:s0 + m]
                eqc = (dv[:, None] == np.arange(P)[None, :]).astype(np.float32)
                eqc = eqc.reshape(n, P, P)  # [chunk, edge, dst]
                for b in range(n):
                    blk = eqc[b]
                    eqA[:, (kg + b) * P:(kg + b + 1) * P] = blk
                    eqD[:, (kg + b) * P:(kg + b + 1) * P] = blk.T
                kg += n
        idx_tabs.append(np.tile(itab, (8, 1)))
        eqA_tabs.append(eqA.astype(bf))
        eqD_tabs.append(eqD.astype(bf))

    # ---- block-mapped full x table (replicated to every core)
    xfull = np.zeros((c.NPTOT, c.F_IN), np.float32)
    for cc in range(c.NCORES):
        loc = np.arange(c.NL)
        gi = _gidx_map(c, np.full(c.NL, cc), loc)
        xfull[gi] = x[cc * c.NL:(cc + 1) * c.NL]
    xfull = xfull.astype(bf)

    # ---- weights (augmented with U = W.T @ a columns), bf16
    def aug(W, a_s, a_d, H, C):
        WT = W.T.astype(np.float64)
        U_s = np.zeros((WT.shape[0], H))
        U_d = np.zeros((WT.shape[0], H))
        for h in range(H):
            U_s[:, h] = WT[:, h * C:(h + 1) * C] @ a_s[h].astype(np.float64)
            U_d[:, h] = WT[:, h * C:(h + 1) * C] @ a_d[h].astype(np.float64)
        return np.concatenate([WT, U_s, U_d], axis=1).astype(bf)

    W1a = aug(W1, a_src1, a_dst1, c.HEADS, c.C1)   # [F_IN, CO1+2H]
    W2a = aug(W2, a_src2, a_dst2, c.HEADS, c.C2)   # [CO1, CO2+2H]
    W3a = aug(W3, a_src3, a_dst3, 1, c.NCLS)       # [CO2, NCLS+2]

    def bln(b, g, be):
        row = np.concatenate([b, g, be]).astype(np.float32)[None, :]
        return np.repeat(row, P, axis=0)

    bln1 = bln(b1, ln1_g, ln1_b)
    bln2 = bln(b2, ln2_g, ln2_b)
    bln3 = bln(b3, ln3_g, ln3_b)

    ident = np.eye(P, dtype=bf)

    in_maps = []
    for cc in range(c.NCORES):
        xl = np.zeros((c.NLP, c.F_IN), np.float32)
        xl[:c.NL] = x[cc * c.NL:(cc + 1) * c.NL]
        in_maps.append({
            "xT": np.ascontiguousarray(xl.T).astype(bf),
            "xfull": xfull,
            "W1a": W1a, "W2a": W2a, "W3a": W3a,
            "bln1": bln1, "bln2": bln2, "bln3": bln3,
            "idx16": idx_tabs[cc],
            "eqA": eqA_tabs[cc], "eqD": eqD_tabs[cc],
            "ident": ident,
        })
    return in_maps, meta


# --------------------------------------------------------------------------
# device program
# --------------------------------------------------------------------------

def build_nc(cfg: Cfg, meta: Meta):
    c = cfg
    H = c.HEADS
    nc = bacc.Bacc("TRN2", target_bir_lowering=False, debug=False,
                   num_devices=c.NCORES, enable_partition_id=False)

    # ---- I/O
    xT = nc.dram_tensor("xT", [c.F_IN, c.NLP], BF16, kind="ExternalInput").ap()
    xfull = nc.dram_tensor("xfull", [c.NPTOT, c.F_IN], BF16,
                           kind="ExternalInput").ap()
    W1a = nc.dram_tensor("W1a", [c.F_IN, c.W1w], BF16, kind="ExternalInput").ap()
    W2a = nc.dram_tensor("W2a", [c.CO1, c.W2w], BF16, kind="ExternalInput").ap()
    W3a = nc.dram_tensor("W3a", [c.CO2, c.W3w], BF16, kind="ExternalInput").ap()
    bln1 = nc.dram_tensor("bln1", [P, 3 * c.CO1], F32, kind="ExternalInput").ap()
    bln2 = nc.dram_tensor("bln2", [P, 3 * c.CO2], F32, kind="ExternalInput").ap()
    bln3 = nc.dram_tensor("bln3", [P, 3 * c.NCLS], F32, kind="ExternalInput").ap()
    idx16 = nc.dram_tensor("idx16", [P, meta.SI], I16, kind="ExternalInput").ap()
    eqA = nc.dram_tensor("eqA", [P, meta.SCE * P], BF16,
                         kind="ExternalInput").ap()
    eqD = nc.dram_tensor("eqD", [P, meta.SCE * P], BF16,
                         kind="ExternalInput").ap()
    ident = nc.dram_tensor("ident", [P, P], BF16, kind="ExternalInput").ap()
    y = nc.dram_tensor("y", [c.NLP, c.NCLS], F32, kind="ExternalOutput").ap()

    groups = [list(range(c.NCORES))]

    with tile.TileContext(nc) as tc:
        dram_cm = tc.tile_pool(name="dram", bufs=1, space="DRAM")
        dram = dram_cm.__enter__()
        hs2_loc = dram.tile([c.NLP, c.ELEM2], BF16)
        hs2_full = [dram.tile([c.GROWS[b], c.ELEM2], BF16, addr_space="Shared",
                              name=f"hs2f{b}") for b in range(c.NG)]
        hs3_loc = dram.tile([c.NLP, c.ELEM3], BF16)
        hs3_full = [dram.tile([c.GROWS[b], c.ELEM3], BF16, addr_space="Shared",
                              name=f"hs3f{b}") for b in range(c.NG)]

        def mk_ag(loc_t, full_ts):
            def fire(b):
                r0 = c.SBT[b] * P
                nr = c.BLK_T[b] * P
                nc.gpsimd.collective_compute(
                    "AllGather", AL.bypass, replica_groups=groups,
                    ins=[loc_t[r0:r0 + nr, :].opt()],
                    outs=[full_ts[b][:].opt()])
            return fire

        ag2 = mk_ag(hs2_loc, hs2_full)
        ag3 = mk_ag(hs3_loc, hs3_full)

        # ---- persistent SBUF constants
        cpool_cm = tc.tile_pool(name="const", bufs=1)
        cpool = cpool_cm.__enter__()
        KC1 = c.F_IN // P
        W1a_sb = cpool.tile([P, KC1 * c.W1w], BF16)
        for k in range(KC1):
            nc.sync.dma_start(W1a_sb[:, k * c.W1w:(k + 1) * c.W1w],
                              W1a[k * P:(k + 1) * P, :])
        KC2 = c.CO1 // P
        W2a_sb = cpool.tile([P, KC2 * c.W2w], BF16)
        for k in range(KC2):
            nc.sync.dma_start(W2a_sb[:, k * c.W2w:(k + 1) * c.W2w],
                              W2a[k * P:(k + 1) * P, :])
        KC3 = c.CO2 // P
        W3a_sb = cpool.tile([P, KC3 * c.W3w], BF16)
        for k in range(KC3):
            nc.sync.dma_start(W3a_sb[:, k * c.W3w:(k + 1) * c.W3w],
                              W3a[k * P:(k + 1) * P, :])
        bln1_sb = cpool.tile([P, 3 * c.CO1], F32)
        nc.sync.dma_start(bln1_sb[:], bln1[:])
        bln2_sb = cpool.tile([P, 3 * c.CO2], F32)
        nc.sync.dma_start(bln2_sb[:], bln2[:])
        bln3_sb = cpool.tile([P, 3 * c.NCLS], F32)
        nc.sync.dma_start(bln3_sb[:], bln3[:])
        idx_sb = cpool.tile([P, meta.SI], I16)
        nc.sync.dma_start(idx_sb[:], idx16[:])
        id_sb = cpool.tile([P, P], BF16)
        nc.sync.dma_start(id_sb[:], ident[:])
        # persistent per-layer local tables: d (dst logits), s1 (self src)
        d1_sb = cpool.tile([P, c.T * H], BF16)
        s1_sb = cpool.tile([P, c.T * H], F32)
        d2_sb = cpool.tile([P, c.T * H], BF16)
        d3_sb = cpool.tile([P, c.T * 1], BF16)

        # ============ prologue: local s1/d1 logits
        with (
            tc.tile_pool(name="pro", bufs=3) as pro,
            tc.tile_pool(name="prop", bufs=2, space="PSUM") as prop,
        ):
            for t in range(c.T):
                xt = pro.tile([P, KC1 * P], BF16, tag="xt")
                for k in range(KC1):
                    nc.sync.dma_start(xt[:, k * P:(k + 1) * P],
                                      xT[k * P:(k + 1) * P, t * P:(t + 1) * P])
                dsp = prop.tile([P, 2 * H], F32, tag="dsp")
                for k in range(KC1):
                    nc.tensor.matmul(
                        out=dsp[:],
                        lhsT=xt[:, k * P:(k + 1) * P],
                        rhs=W1a_sb[:, k * c.W1w + c.CO1:
                                   k * c.W1w + c.CO1 + 2 * H],
                        start=(k == 0), stop=(k == KC1 - 1))
                nc.vector.tensor_copy(s1_sb[:, t * H:(t + 1) * H],
                                      dsp[:, 0:H])
                nc.vector.tensor_copy(d1_sb[:, t * H:(t + 1) * H],
                                      dsp[:, H:2 * H])

        # ============ layer 1 (per-edge transform, fused L2 transform)
        _l1_phase(nc, tc, c, meta, xT, xfull, W1a_sb, W2a_sb, bln1_sb,
                  id_sb, idx_sb, eqA, eqD, d1_sb, s1_sb, d2_sb, hs2_loc, ag2)

        # ============ layer 2 (+fused L3 transform)
        _edge_phase(
            nc, tc, c, meta, lay=2, Hn=H, Ch=c.C2, ELEM=c.ELEM2,
            hs_loc=hs2_loc, hs_full=hs2_full, d_sb=d2_sb, bln_sb=bln2_sb,
            id_sb=id_sb, idx_sb=idx_sb, eqA=eqA, eqD=eqD,
            fuse=dict(W_sb=W3a_sb, KC=KC3, Ww=c.W3w, CO=c.NCLS, Hn2=1,
                      ELEMn=c.ELEM3, hs_loc=hs3_loc, d_next=d3_sb),
            final=None, y=None, ag_fire=ag3)

        # ============ layer 3 + log_softmax
        _edge_phase(
            nc, tc, c, meta, lay=3, Hn=1, Ch=c.NCLS, ELEM=c.ELEM3,
            hs_loc=hs3_loc, hs_full=hs3_full, d_sb=d3_sb, bln_sb=bln3_sb,
            id_sb=id_sb, idx_sb=idx_sb, eqA=eqA, eqD=eqD,
            fuse=None, final=True, y=y, ag_fire=None)

        cpool_cm.__exit__(None, None, None)
        dram_cm.__exit__(None, None, None)

    nc.compile()
    return nc


# --------------------------------------------------------------------------
# layer-1 phase: per-edge transform with wf-scaled PSUM->SBUF copies
# --------------------------------------------------------------------------

def _l1_phase(nc, tc, c: Cfg, meta: Meta, xT, xfull, W1a_sb, W2a_sb, bln_sb,
              id_sb, idx_sb, eqA, eqD, d1_sb, s1_sb, d2_sb, hs2_loc, ag_fire):
    H = c.HEADS
    CO = c.CO1
    Ch = c.C1
    nch = meta.nch
    ntot = meta.ntot
    max_nch = max(max(r) for r in nch)
    max_ntot = max(ntot)
    KC1 = c.F_IN // P

    with (
        tc.tile_pool(name="gx", bufs=3) as gxp,
        tc.tile_pool(name="xt1", bufs=3) as xtp,
        tc.tile_pool(name="eqs", bufs=3) as eqp,
        tc.tile_pool(name="wch", bufs=2) as wcp,
        tc.tile_pool(name="gp1", bufs=2) as gpp,
        tc.tile_pool(name="ep", bufs=1) as ep,
        tc.tile_pool(name="hst1", bufs=2) as hstp,
        tc.tile_pool(name="psh", bufs=2, space="PSUM") as psh,
        tc.tile_pool(name="psagg", bufs=1, space="PSUM") as psagg,
        tc.tile_pool(name="pssd", bufs=2, space="PSUM") as pssd,
        tc.tile_pool(name="pscr", bufs=1, space="PSUM") as pscr,
    ):
        state = {}

        def stage_fetch(t):
            st = state.setdefault(t, {})
            # gathers (Pool)
            for g in range(c.NG):
                nb = nch[t][g]
                if nb == 0:
                    continue
                si = meta.sig[g][t]
                nidx = nb * P
                Gx = gxp.tile([P, KC1 * max_nch * P], BF16, tag=f"Gx{g}")
                nc.gpsimd.dma_gather(
                    out_ap=Gx[:, 0:KC1 * nidx].rearrange(
                        "p (j e) -> p j e", e=nidx),
                    in_ap=xfull[c.GBASE[g]:c.GBASE[g] + c.GROWS[g], :],
                    idxs_ap=idx_sb[:, si:si + nb * (P // 16)],
                    num_idxs=nidx, num_idxs_reg=nidx, elem_size=c.F_IN,
                    transpose=True)
                st.setdefault("Gx", {})[g] = Gx
            # self xT tile (HWDGE)
            xt = xtp.tile([P, KC1 * P], BF16, tag="xt")
            for k in range(KC1):
                nc.sync.dma_start(xt[:, k * P:(k + 1) * P],
                                  xT[k * P:(k + 1) * P, t * P:(t + 1) * P])
            st["xt"] = xt
            # eq / eqd streams (HWDGE sync + scalar queues)
            nt = ntot[t]
            e0 = meta.sce[t] * P
            eqa_sb = eqp.tile([P, max_ntot * P], BF16, tag="eqa")
            nc.sync.dma_start(eqa_sb[:, 0:nt * P], eqA[:, e0:e0 + nt * P])
            eqd_sb = eqp.tile([P, max_ntot * P], BF16, tag="eqd")
            nc.scalar.dma_start(eqd_sb[:, 0:nt * P], eqD[:, e0:e0 + nt * P])
            st["eqa"] = eqa_sb
            st["eqd"] = eqd_sb

        def lhsT_k(st, t, k, j):
            """stationary x^T block (feature subtile j) for chunk k."""
            if k == 0:
                return st["xt"][:, j * P:(j + 1) * P]
            b = k - 1
            n0 = nch[t][0]
            g = 0 if b < n0 else 1
            if g == 1:
                b -= n0
            Gx = st["Gx"][g]
            nidx = nch[t][g] * P
            return Gx[:, 0:KC1 * nidx].rearrange(
                "p (j e) -> p j e", e=nidx)[:, j, b * P:(b + 1) * P]

        DOFF = 128            # d-logit section (f32 cols) in the sd bank
        DENOFF = 384          # den section
        SDNOFF = 400          # fused next-layer s|d section
        assert max_ntot * H <= DOFF

        def stage_logits(t):
            """per-chunk s and d logits -> wf (exp of leaky)."""
            st = state[t]
            nt = ntot[t]
            eqd_sb = st["eqd"]
            sd = pssd.tile([P, 512], F32, tag="sd")
            st["sd"] = sd
            DO = DOFF
            # d logits: eqd chunk @ d1 column
            for k in range(nt):
                nc.tensor.matmul(
                    out=sd[:, DO + k * H:DO + (k + 1) * H],
                    lhsT=eqd_sb[:, k * P:(k + 1) * P],
                    rhs=d1_sb[:, t * H:(t + 1) * H],
                    start=True, stop=True, skip_group_check=True)
            # s logits for gathered chunks (self comes from s1_sb)
            for k in range(1, nt):
                for j in range(KC1):
                    nc.tensor.matmul(
                        out=sd[:, k * H:(k + 1) * H],
                        lhsT=lhsT_k(st, t, k, j),
                        rhs=W1a_sb[:, j * c.W1w + CO:j * c.W1w + CO + H],
                        start=(j == 0), stop=(j == KC1 - 1),
                        skip_group_check=True)
            # tsd = s + d ; leaky ; exp  (DVE reads at most one PSUM input:
            # stage the s section through SBUF first)
            ssb = wcp.tile([P, max_ntot * H], F32, tag="ssb")
            nc.vector.tensor_copy(ssb[:, 0:H], s1_sb[:, t * H:(t + 1) * H])
            if nt > 1:
                nc.vector.tensor_copy(ssb[:, H:nt * H], sd[:, H:nt * H])
            tsd = wcp.tile([P, max_ntot * H], F32, tag="tsd")
            nc.vector.tensor_tensor(
                out=tsd[:, 0:nt * H], in0=ssb[:, 0:nt * H],
                in1=sd[:, DO:DO + nt * H], op=AL.add)
            lra = wcp.tile([P, max_ntot * H], F32, tag="lra")
            nc.vector.scalar_tensor_tensor(
                out=lra[:, 0:nt * H], in0=tsd[:, 0:nt * H],
                scalar=NEG_SLOPE_ATT, in1=tsd[:, 0:nt * H],
                op0=AL.mult, op1=AL.max)
            wfa = wcp.tile([P, max_ntot * H], F32, tag="wfa")
            nc.scalar.activation(wfa[:, 0:nt * H], lra[:, 0:nt * H], AF.Exp)
            wfb = wcp.tile([P, max_ntot * H], BF16, tag="wfb")
            nc.vector.tensor_copy(wfb[:, 0:nt * H], wfa[:, 0:nt * H])
            st["wfa"] = wfa
            st["wfb"] = wfb

        def stage_agg(t):
            """per chunk: transform -> wf-scaled copy -> agg/den matmuls."""
            st = state[t]
            nt = ntot[t]
            eqa_sb = st["eqa"]
            wfa, wfb = st["wfa"], st["wfb"]
            aggA = psagg.tile([P, 512], F32, tag="aggA")
            aggB = psagg.tile([P, 512], F32, tag="aggB")
            den = st["sd"][:, DENOFF:DENOFF + H]
            st["agg"] = (aggA, aggB)
            for k in range(nt):
                first, last = (k == 0), (k == nt - 1)
                Gp = gpp.tile([P, CO], BF16, tag="gp")
                for half in range(2):
                    hp = psh.tile([P, 512], F32, tag="hp")
                    n0 = half * 512
                    for j in range(KC1):
                        nc.tensor.matmul(
                            out=hp[:],
                            lhsT=lhsT_k(st, t, k, j),
                            rhs=W1a_sb[:, j * c.W1w + n0:
                                       j * c.W1w + n0 + 512],
                            start=(j == 0), stop=(j == KC1 - 1))
                    # wf-scaled PSUM->SBUF copy (2 heads per half)
                    for hh in range(2):
                        h = half * 2 + hh
                        dst = Gp[:, n0 + hh * Ch:n0 + (hh + 1) * Ch]
                        src = hp[:, hh * Ch:(hh + 1) * Ch]
                        sc = wfa[:, k * H + h:k * H + h + 1]
                        if hh == 0:
                            nc.scalar.activation(dst, src, AF.Copy, scale=sc)
                        else:
                            nc.vector.tensor_scalar(
                                out=dst, in0=src, scalar1=sc,
                                scalar2=None, op0=AL.mult)
                eq = eqa_sb[:, k * P:(k + 1) * P]
                nc.tensor.matmul(out=aggA[:], lhsT=eq, rhs=Gp[:, 0:512],
                                 start=first, stop=last,
                                 skip_group_check=True)
                nc.tensor.matmul(out=aggB[:], lhsT=eq, rhs=Gp[:, 512:1024],
                                 start=first, stop=last,
                                 skip_group_check=True)
                nc.tensor.matmul(out=den, lhsT=eq,
                                 rhs=wfb[:, k * H:(k + 1) * H],
                                 start=first, stop=last,
                                 skip_group_check=True)

        def stage_epi(t):
            st = state[t]
            aggA, aggB = st["agg"]
            agg_aps = [aggA[:, 0:Ch], aggA[:, Ch:2 * Ch],
                       aggB[:, 0:Ch], aggB[:, Ch:2 * Ch]]
            den_ap = st["sd"][:, DENOFF:DENOFF + H]
            sdn_ap = st["sd"][:, SDNOFF:SDNOFF + 2 * c.HEADS]
            _epilogue_ln(nc, ep, pscr, psh, c, t, agg_aps,
                         den_ap, H, Ch, CO, bln_sb, id_sb,
                         fuse=dict(W_sb=W2a_sb, KC=CO // P, Ww=c.W2w,
                                   CO=c.CO2, Hn2=c.HEADS, ELEMn=c.ELEM2,
                                   hs_loc=hs2_loc, d_next=d2_sb,
                                   hstp=hstp, sdn_ap=sdn_ap),
                         final=False, y=None)
            del state[t]
            if ag_fire is not None:
                for b in range(c.NG):
                    if t == c.SBT[b] + c.BLK_T[b] - 1:
                        ag_fire(b)

        stage_fetch(0)
        stage_fetch(1)
        stage_logits(0)
        for t in range(c.T):
            if t + 2 < c.T:
                stage_fetch(t + 2)
            if t + 1 < c.T:
                stage_logits(t + 1)
            stage_agg(t)
            stage_epi(t)


# --------------------------------------------------------------------------
# layers 2/3: table-gather edge phase
# --------------------------------------------------------------------------

def _edge_phase(nc, tc, c: Cfg, meta: Meta, lay, Hn, Ch, ELEM, hs_loc,
                hs_full, d_sb, bln_sb, id_sb, idx_sb, eqA, eqD,
                fuse, final, y, ag_fire):
    CO = Hn * Ch
    nch = meta.nch
    ntot = meta.ntot
    max_nch = max(max(r) for r in nch)
    max_ntot = max(ntot)
    merge_den = (Hn == 1)

    with (
        tc.tile_pool(name=f"g{lay}", bufs=3) as gp,
        tc.tile_pool(name=f"gs{lay}", bufs=3) as gsp,
        tc.tile_pool(name=f"eq{lay}", bufs=3) as eqp,
        tc.tile_pool(name=f"wc{lay}", bufs=2) as wcp,
        tc.tile_pool(name=f"gp{lay}", bufs=3) as gpp,
        tc.tile_pool(name=f"ep{lay}", bufs=1) as ep,
        tc.tile_pool(name=f"hs{lay}", bufs=2) as hstp,
        tc.tile_pool(name=f"psa{lay}", bufs=2, space="PSUM") as psagg,
        tc.tile_pool(name=f"psd{lay}", bufs=2, space="PSUM") as pssd,
        tc.tile_pool(name=f"psh{lay}", bufs=2, space="PSUM") as psh,
        tc.tile_pool(name=f"psc{lay}", bufs=1, space="PSUM") as pscr,
    ):
        state = {}

        def stage_fetch(t, groups=(0, 1), do_rest=True):
            st = state.setdefault(t, {})
            for g in groups:
                nb = nch[t][g]
                if nb == 0:
                    continue
                si = meta.sig[g][t]
                nidx = nb * P
                G = gp.tile([P, max_nch * ELEM], BF16, tag=f"G{g}")
                nc.gpsimd.dma_gather(
                    out_ap=G[:, 0:nb * ELEM].rearrange(
                        "p (k d) -> p k d", d=ELEM),
                    in_ap=hs_full[g][:],
                    idxs_ap=idx_sb[:, si:si + nb * (P // 16)],
                    num_idxs=nidx, num_idxs_reg=nidx, elem_size=ELEM)
                st.setdefault("G", {})[g] = G
            if not do_rest:
                return
            Gs = gsp.tile([P, ELEM], BF16, tag="Gs")
            nc.sync.dma_start(Gs[:], hs_loc[t * P:(t + 1) * P, :])
            st["Gself"] = Gs
            nt = ntot[t]
            e0 = meta.sce[t] * P
            eqa_sb = eqp.tile([P, max_ntot * P], BF16, tag="eqa")
            nc.sync.dma_start(eqa_sb[:, 0:nt * P], eqA[:, e0:e0 + nt * P])
            eqd_sb = eqp.tile([P, max_ntot * P], BF16, tag="eqd")
            nc.scalar.dma_start(eqd_sb[:, 0:nt * P], eqD[:, e0:e0 + nt * P])
            st["eqa"] = eqa_sb
            st["eqd"] = eqd_sb

        DENOFF = 384          # den section (f32 cols) in the dps bank
        SDNOFF = 400          # fused next-layer s|d section
        assert max_ntot * Hn <= DENOFF

        def stage_logits(t):
            st = state[t]
            nt = ntot[t]
            eqd_sb = st["eqd"]
            dps = pssd.tile([P, 512], F32, tag="dps")
            st["dps"] = dps
            for k in range(nt):
                nc.tensor.matmul(
                    out=dps[:, k * Hn:(k + 1) * Hn],
                    lhsT=eqd_sb[:, k * P:(k + 1) * P],
                    rhs=d_sb[:, t * Hn:(t + 1) * Hn],
                    start=True, stop=True, skip_group_check=True)
            tsd = wcp.tile([P, max_ntot * Hn], F32, tag="tsd")
            nc.vector.tensor_tensor(
                out=tsd[:, 0:Hn], in0=st["Gself"][:, CO:CO + Hn],
                in1=dps[:, 0:Hn], op=AL.add)
            b0 = 1
            for g in range(c.NG):
                n = nch[t][g]
                if n == 0:
                    continue
                Gv = st["G"][g][:, 0:n * ELEM].rearrange(
                    "p (k d) -> p k d", d=ELEM)[:, :, CO:CO + Hn]
                Dv = dps[:, b0 * Hn:(b0 + n) * Hn].rearrange(
                    "p (k h) -> p k h", h=Hn)
                nc.vector.tensor_tensor(
                    out=tsd[:, b0 * Hn:(b0 + n) * Hn].rearrange(
                        "p (k h) -> p k h", h=Hn),
                    in0=Gv, in1=Dv, op=AL.add)
                b0 += n
            lra = wcp.tile([P, max_ntot * Hn], F32, tag="lra")
            nc.vector.scalar_tensor_tensor(
                out=lra[:, 0:nt * Hn], in0=tsd[:, 0:nt * Hn],
                scalar=NEG_SLOPE_ATT, in1=tsd[:, 0:nt * Hn],
                op0=AL.mult, op1=AL.max)
            wfa = wcp.tile([P, max_ntot * Hn], F32, tag="wfa")
            nc.scalar.activation(wfa[:, 0:nt * Hn], lra[:, 0:nt * Hn], AF.Exp)
            st["wfa"] = wfa
            if not merge_den:
                wfb = wcp.tile([P, max_ntot * Hn], BF16, tag="wfb")
                nc.vector.tensor_copy(wfb[:, 0:nt * Hn], wfa[:, 0:nt * Hn])
                st["wfb"] = wfb

        def G_rows(st, t, k):
            if k == 0:
                return st["Gself"], 0
            b = k - 1
            n0 = nch[t][0]
            g = 0 if b < n0 else 1
            if g == 1:
                b -= n0
            return st["G"][g], b * ELEM

        def stage_agg(t):
            st = state[t]
            nt = ntot[t]
            eqa_sb = st["eqa"]
            wfa = st["wfa"]
            NAGG = CO + (1 if merge_den else 0)
            agg = psagg.tile([P, NAGG], F32, tag="agg")
            st["agg"] = agg
            if merge_den:
                den_ap = agg[:, CO:CO + 1]
            else:
                den_ap = st["dps"][:, DENOFF:DENOFF + Hn]
            st["den_ap"] = den_ap
            for k in range(nt):
                first, last = (k == 0), (k == nt - 1)
                Gt, off = G_rows(st, t, k)
                Gp = gpp.tile([P, NAGG], BF16, tag="gp")
                nc.vector.tensor_tensor(
                    out=Gp[:, 0:CO].rearrange("p (h d) -> p h d", h=Hn),
                    in0=Gt[:, off:off + CO].rearrange(
                        "p (h d) -> p h d", h=Hn),
                    in1=wfa[:, k * Hn:(k + 1) * Hn].to_broadcast(
                        [P, Hn, Ch]),
                    op=AL.mult)
                if merge_den:
                    nc.vector.tensor_copy(Gp[:, CO:CO + 1],
                                          wfa[:, k:k + 1])
                eq = eqa_sb[:, k * P:(k + 1) * P]
                for n0 in range(0, NAGG, 512):
                    nsz = min(512, NAGG - n0)
                    nc.tensor.matmul(out=agg[:, n0:n0 + nsz], lhsT=eq,
                                     rhs=Gp[:, n0:n0 + nsz],
                                     start=first, stop=last,
                                     skip_group_check=True)
                if not merge_den:
                    nc.tensor.matmul(out=den_ap, lhsT=eq,
                                     rhs=st["wfb"][:, k * Hn:(k + 1) * Hn],
                                     start=first, stop=last,
                                     skip_group_check=True)

        def stage_epi(t):
            st = state[t]
            agg = st["agg"]
            agg_aps = [agg[:, h * Ch:(h + 1) * Ch] for h in range(Hn)]
            fz = None
            if fuse is not None:
                fz = dict(fuse)
                fz["hstp"] = hstp
                fz["sdn_ap"] = st["dps"][:, SDNOFF:
                                         SDNOFF + 2 * fuse["Hn2"]]
            _epilogue_ln(nc, ep, pscr, psh, c, t, agg_aps,
                         st["den_ap"], Hn, Ch, CO, bln_sb, id_sb,
                         fuse=fz, final=final, y=y)
            del state[t]
            if ag_fire is not None:
                for b in range(c.NG):
                    if t == c.SBT[b] + c.BLK_T[b] - 1:
                        ag_fire(b)

        stage_fetch(0, groups=(0,), do_rest=True)
        stage_fetch(1, groups=(0,), do_rest=True)
        stage_fetch(0, groups=(1,), do_rest=False)
        stage_fetch(1, groups=(1,), do_rest=False)
        stage_logits(0)
        for t in range(c.T):
            if t + 2 < c.T:
                stage_fetch(t + 2)
            if t + 1 < c.T:
                stage_logits(t + 1)
            stage_agg(t)
            stage_epi(t)


# --------------------------------------------------------------------------
# shared epilogue: softmax-normalize, +bias, LayerNorm, leaky / log_softmax,
# optional fused next-layer transform + hs store
# --------------------------------------------------------------------------

def _epilogue_ln(nc, sb, pscr, psh, c, t, agg_aps, den_ap, Hn, Ch, CO,
                 bln_sb, id_sb, fuse, final, y):
    denr = sb.tile([P, Hn], F32, tag="denr")
    nc.vector.tensor_scalar(out=denr[:], in0=den_ap, scalar1=1e-16,
                            scalar2=None, op0=AL.add)
    rec = sb.tile([P, Hn], F32, tag="rec")
    nc.vector.reciprocal(rec[:], denr[:])
    ob = sb.tile([P, CO], F32 if final else BF16, tag="ob")
    for h in range(Hn):
        nc.vector.scalar_tensor_tensor(
            out=ob[:, h * Ch:(h + 1) * Ch], in0=agg_aps[h],
            scalar=rec[:, h:h + 1], op0=AL.mult,
            in1=bln_sb[:, h * Ch:(h + 1) * Ch], op1=AL.add)
    rs = sb.tile([P, 1], F32, tag="rs")
    nc.vector.tensor_reduce(out=rs[:], in_=ob[:], axis=mybir.AxisListType.X,
                            op=AL.add)
    nm = sb.tile([P, 1], F32, tag="nm")
    nc.vector.tensor_scalar(out=nm[:], in0=rs[:], scalar1=-1.0 / CO,
                            scalar2=None, op0=AL.mult)
    sqd = sb.tile([P, CO], BF16, tag="sqd")
    vs = sb.tile([P, 1], F32, tag="vs")
    nc.scalar.activation(sqd[:], ob[:], AF.Square, bias=nm[:, 0:1],
                         accum_out=vs[:])
    vstd = sb.tile([P, 1], F32, tag="vstd")
    nc.vector.tensor_scalar(out=vstd[:], in0=vs[:], scalar1=1.0 / CO,
                            scalar2=LN_EPS, op0=AL.mult, op1=AL.add)
    sd = sb.tile([P, 1], F32, tag="sd")
    nc.scalar.activation(sd[:], vstd[:], AF.Sqrt)
    rstd = sb.tile([P, 1], F32, tag="rstd")
    nc.vector.reciprocal(rstd[:], sd[:])
    xcs = sb.tile([P, CO], F32 if final else BF16, tag="xcs")
    nc.vector.tensor_scalar(out=xcs[:], in0=ob[:], scalar1=nm[:, 0:1],
                            scalar2=rstd[:, 0:1], op0=AL.add, op1=AL.mult)
    y1 = sb.tile([P, CO], F32 if final else BF16, tag="y1")
    nc.vector.tensor_tensor(out=y1[:], in0=xcs[:], in1=bln_sb[:, CO:2 * CO],
                            op=AL.mult)
    y2 = sb.tile([P, CO], F32 if final else BF16, tag="y2")
    nc.vector.tensor_tensor(out=y2[:], in0=y1[:], in1=bln_sb[:, 2 * CO:3 * CO],
                            op=AL.add)

    if final:
        mx = sb.tile([P, 1], F32, tag="mx")
        nc.vector.tensor_reduce(out=mx[:], in_=y2[:],
                                axis=mybir.AxisListType.X, op=AL.max)
        nmx = sb.tile([P, 1], F32, tag="nmx")
        nc.vector.tensor_scalar(out=nmx[:], in0=mx[:], scalar1=-1.0,
                                scalar2=None, op0=AL.mult)
        xs = sb.tile([P, CO], F32, tag="xs")
        nc.vector.tensor_scalar(out=xs[:], in0=y2[:], scalar1=nmx[:, 0:1],
                                scalar2=None, op0=AL.add)
        ex = sb.tile([P, CO], F32, tag="ex")
        se = sb.tile([P, 1], F32, tag="se")
        nc.scalar.activation(ex[:], xs[:], AF.Exp, accum_out=se[:])
        lse = sb.tile([P, 1], F32, tag="lse")
        nc.scalar.activation(lse[:], se[:], AF.Ln)
        nlse = sb.tile([P, 1], F32, tag="nlse")
        nc.vector.tensor_scalar(out=nlse[:], in0=lse[:], scalar1=-1.0,
                                scalar2=None, op0=AL.mult)
        yo = sb.tile([P, CO], F32, tag="yo")
        nc.vector.tensor_scalar(out=yo[:], in0=xs[:], scalar1=nlse[:, 0:1],
                                scalar2=None, op0=AL.add)
        nc.sync.dma_start(y[t * P:(t + 1) * P, :], yo[:])
        return

    x2 = sb.tile([P, CO], BF16, tag="x2")
    nc.vector.scalar_tensor_tensor(
        out=x2[:], in0=y2[:], scalar=NEG_SLOPE_ACT, in1=y2[:],
        op0=AL.mult, op1=AL.max)
    W_sb, KC, Ww = fuse["W_sb"], fuse["KC"], fuse["Ww"]
    CO2, Hn2, ELEMn = fuse["CO"], fuse["Hn2"], fuse["ELEMn"]
    xt2 = sb.tile([P, KC * P], BF16, tag="xt2")
    for k in range(KC):
        scr = pscr.tile([P, P], BF16, tag="scr")
        nc.tensor.transpose(out=scr[:], in_=x2[:, k * P:(k + 1) * P],
                            identity=id_sb[:])
        nc.scalar.copy(xt2[:, k * P:(k + 1) * P], scr[:])
    # next-layer transform: h part (<=512 cols) into psh, s|d into pssd
    NH = CO2  # h columns
    hp = psh.tile([P, 512], F32, tag="hp")
    for k in range(KC):
        nc.tensor.matmul(
            out=hp[:, 0:NH],
            lhsT=xt2[:, k * P:(k + 1) * P],
            rhs=W_sb[:, k * Ww:k * Ww + NH],
            start=(k == 0), stop=(k == KC - 1))
    sdp = fuse["sdn_ap"]
    for k in range(KC):
        nc.tensor.matmul(
            out=sdp,
            lhsT=xt2[:, k * P:(k + 1) * P],
            rhs=W_sb[:, k * Ww + NH:k * Ww + NH + 2 * Hn2],
            start=(k == 0), stop=(k == KC - 1),
            skip_group_check=True)
    hstp = fuse["hstp"]
    hst = hstp.tile([P, ELEMn], BF16, tag="hst")
    nc.scalar.copy(hst[:, 0:CO2], hp[:, 0:CO2])
    nc.vector.tensor_copy(hst[:, CO2:CO2 + Hn2], sdp[:, 0:Hn2])
    if ELEMn > CO2 + Hn2:
        nc.vector.memset(hst[:, CO2 + Hn2:ELEMn], 0)
    nc.vector.tensor_copy(fuse["d_next"][:, t * Hn2:(t + 1) * Hn2],
                          sdp[:, Hn2:2 * Hn2])
    nc.sync.dma_start(fuse["hs_loc"][t * P:(t + 1) * P, :], hst[:])


# --------------------------------------------------------------------------
# entry point
# --------------------------------------------------------------------------

_CACHE = {}


def _get_nc(cfg, meta):
    key = (tuple(sorted((k, str(v)) for k, v in cfg.__dict__.items())),
           tuple(tuple(r) for r in meta.nch))
    if key not in _CACHE:
        _CACHE[key] = build_nc(cfg, meta)
    return _CACHE[key]


def kernel(**inputs):
    inputs = {k: np.asarray(v) for k, v in inputs.items()}
    x = inputs["x"]
    cfg = Cfg(N=x.shape[0], E=inputs["edge_src"].shape[0], F_IN=x.shape[1],
              HEADS=inputs["a_src1"].shape[0], C1=inputs["a_src1"].shape[1],
              C2=inputs["a_src2"].shape[1], NCLS=inputs["W3"].shape[0],
              NCORES=8)
    in_maps, meta = host_prep(cfg, **inputs)
    nc = _get_nc(cfg, meta)
    trace = bool(int(os.environ.get("GAT_TRACE", "0")))
    res = run_bass_kernel_spmd(nc, in_maps, core_ids=list(range(cfg.NCORES)),
                               trace=trace)
    global LAST_EXEC_NS, LAST_RES
    LAST_EXEC_NS = res.exec_time_ns
    LAST_RES = res
    out = np.concatenate(
        [res.results[cc]["y"][:cfg.NL] for cc in range(cfg.NCORES)], axis=0)
    return out.astype(np.float32)


LAST_EXEC_NS = None
LAST_RES = None


if __name__ == "__main__":
    pass
